# revision 2
# baseline (speedup 1.0000x reference)
"""Multi-head attention forward on 8 Trainium2 NeuronCores.

Sharding: core = (batch b in 0..2, head-group hg in 0..4); each core owns
4 of the 16 heads for one batch element. Q/K/V projections are computed
per-core for its 256 head-dims; attention runs per head with scores kept
transposed (S^T[k, q]) so no on-chip transposes are needed; the output
projection is row-sharded over W_o, producing a per-core partial Y that
the host sums over the 4 head-groups of each batch.
"""

import sys

for _p in ("/opt/trn_rl_repo", "/opt/pypackages"):
    if _p not in sys.path:
        sys.path.append(_p)

from contextlib import ExitStack

import numpy as np

import concourse.bass as bass
import concourse.tile as tile
from concourse import bacc, mybir
from concourse import bass_utils

P = 128
B = 2
S = 2048          # sequence length
D = 1024          # model dim
H = 16            # total heads
DK = 64           # head dim
HL = 4            # heads per core
CL = HL * DK      # local head dims per core (256)
NJ = 4            # 512-wide s-slices
NS = 512
NI = D // P       # 8 contraction tiles over model dim
NK = S // P       # 16 key tiles
NQB = S // P      # 16 query blocks for the output projection

F32 = mybir.dt.float32
F32R = mybir.dt.float32r
EXP = mybir.ActivationFunctionType.Exp

# k-tile groups per (head, q-slice): scores for a group land in one PSUM
# tile and get one big exp instruction ([128, len*512] PSUM -> SBUF).
K_GROUPS = [(0, 1, 2), (3, 4, 5), (6, 7, 8), (9, 10, 11), (12, 13, 14), (15,)]


def _r(ap):
    return ap.bitcast(F32R)


def build_nc():
    nc = bacc.Bacc("TRN2", target_bir_lowering=False, debug=False)

    xqT = nc.dram_tensor("xqT", [D, S], F32, kind="ExternalInput")
    xkT = nc.dram_tensor("xkT", [D, S], F32, kind="ExternalInput")
    xvT = nc.dram_tensor("xvT", [D, S], F32, kind="ExternalInput")
    wqT = nc.dram_tensor("wqT", [D, CL], F32, kind="ExternalInput")
    wkT = nc.dram_tensor("wkT", [D, CL], F32, kind="ExternalInput")
    wvT = nc.dram_tensor("wvT", [D, CL], F32, kind="ExternalInput")
    woT = nc.dram_tensor("woT", [CL, D], F32, kind="ExternalInput")
    y = nc.dram_tensor("y", [S, D], F32, kind="ExternalOutput")

    with tile.TileContext(nc) as tc, ExitStack() as ctx:
        wpool = ctx.enter_context(tc.tile_pool(name="w", bufs=1))
        big = ctx.enter_context(tc.tile_pool(name="big", bufs=1))

        # Resident weights
        wq_sb = wpool.tile([P, NI, CL], F32R)
        wk_sb = wpool.tile([P, NI, CL], F32R)
        wv_sb = wpool.tile([P, NI, CL], F32R)
        wo_sb = wpool.tile([P, CL // P, D], F32R)
        nc.sync.dma_start(wq_sb[:], wqT.ap().rearrange("(n p) o -> p n o", p=P).bitcast(F32R))
        nc.sync.dma_start(wk_sb[:], wkT.ap().rearrange("(n p) o -> p n o", p=P).bitcast(F32R))
        nc.sync.dma_start(wv_sb[:], wvT.ap().rearrange("(n p) o -> p n o", p=P).bitcast(F32R))
        nc.sync.dma_start(wo_sb[:], woT.ap().rearrange("(n p) o -> p n o", p=P).bitcast(F32R))

        # Resident activations: Q^T / K^T with head dims on partitions
        # ([128, o-tile, s]); V in natural [s, d] layout with a ones column
        # appended per head (for softmax denominators); normalized ctx^T.
        qT_sb = big.tile([P, 2, S], F32R)
        kT_sb = big.tile([P, 2, S], F32R)
        v_sb = big.tile([P, NK, HL * 65], F32R)
        cT_sb = big.tile([P, 2, S], F32R)

        nc.vector.memset(v_sb[:].bitcast(F32), 1.0)

        # ---- Phase A: projections -------------------------------------
        with tc.tile_pool(name="xs", bufs=12) as xpool, \
             tc.tile_pool(name="psA", bufs=2, space="PSUM") as psA, \
             tc.tile_pool(name="psV", bufs=2, space="PSUM") as psV:

            def proj_T(x_dram, w_sb, out_sb):
                # out_sb[:, ot, j*512:...] = (X @ W.T)^T for this core's dims
                for j in range(NJ):
                    xt = []
                    for i in range(NI):
                        t = xpool.tile([P, NS], F32R, tag="x")
                        nc.sync.dma_start(
                            t[:],
                            x_dram.ap()[i * P:(i + 1) * P, j * NS:(j + 1) * NS].bitcast(F32R),
                        )
                        xt.append(t)
                    for ot in range(2):
                        ps = psA.tile([P, NS], F32, tag="ps")
                        for i in range(NI):
                            nc.tensor.matmul(
                                ps[:],
                                wq := w_sb[:, i, ot * P:(ot + 1) * P],
                                xt[i][:],
                                start=(i == 0),
                                stop=(i == NI - 1),
                            )
                        nc.scalar.copy(out_sb[:, ot, j * NS:(j + 1) * NS], ps[:])

            proj_T(xkT, wk_sb, kT_sb)

            # V in natural layout: out[s_tile] = X_v @ W_v.T
            for j in range(NJ):
                xt = []
                for i in range(NI):
                    t = xpool.tile([P, NS], F32R, tag="x")
                    nc.sync.dma_start(
                        t[:],
                        xvT.ap()[i * P:(i + 1) * P, j * NS:(j + 1) * NS].bitcast(F32R),
                    )
                    xt.append(t)
                for sb in range(4):
                    psv = psV.tile([P, CL], F32, tag="psv")
                    for i in range(NI):
                        nc.tensor.matmul(
                            psv[:],
                            xt[i][:, sb * P:(sb + 1) * P],
                            wv_sb[:, i, :],
                            start=(i == 0),
                            stop=(i == NI - 1),
                        )
                    st = j * 4 + sb
                    nc.vector.tensor_copy(
                        v_sb[:, st].rearrange("p (h c) -> p h c", c=65)[:, :, 0:64],
                        psv.rearrange("p (h c) -> p h c", c=64)[:],
                    )

            proj_T(xqT, wq_sb, qT_sb)

        # ---- Phase B: attention ---------------------------------------
        with tc.tile_pool(name="ex", bufs=3) as epool, \
             tc.tile_pool(name="sm", bufs=3) as spool, \
             tc.tile_pool(name="psS", bufs=2, space="PSUM") as psS, \
             tc.tile_pool(name="psB", bufs=2, space="PSUM") as psB:
            for h in range(HL):
                pr0 = (h % 2) * 64
                ot = h // 2
                vcol = h * 65
                for j in range(NJ):
                    ctx_ps = psB.tile([65, NS], F32, tag="ctx")
                    for ks in K_GROUPS:
                        ng = len(ks)
                        sp = psS.tile([P, 3, NS], F32, tag="sc")
                        for idx, k in enumerate(ks):
                            nc.tensor.matmul(
                                sp[:, idx],
                                kT_sb[pr0:pr0 + 64, ot, k * P:(k + 1) * P],
                                qT_sb[pr0:pr0 + 64, ot, j * NS:(j + 1) * NS],
                                start=True,
                                stop=True,
                            )
                        ex = epool.tile([P, 3, NS], F32R, tag="ex")
                        nc.scalar.activation(
                            ex[:, 0:ng], sp[:, 0:ng], EXP, scale=0.125,
                        )
                        for idx, k in enumerate(ks):
                            nc.tensor.matmul(
                                ctx_ps[:],
                                v_sb[:, k, vcol:vcol + 65],
                                ex[:, idx],
                                start=(k == 0),
                                stop=(k == NK - 1),
                            )
                    rec = spool.tile([1, NS], F32, tag="rec")
                    nc.vector.reciprocal(rec[:], ctx_ps[64:65, :])
                    bc = spool.tile([64, NS], F32, tag="bc")
                    nc.gpsimd.partition_broadcast(bc[:], rec[:])
                    dst = cT_sb[pr0:pr0 + 64, ot, j * NS:(j + 1) * NS]
                    if h % 2 == 0:
                        nc.vector.tensor_mul(dst, ctx_ps[0:64, :], bc[:])
                    else:
                        tmp = spool.tile([64, NS], F32R, tag="tmp")
                        nc.vector.tensor_mul(tmp[:], ctx_ps[0:64, :], bc[:])
                        nc.sync.dma_start(dst, tmp[:])

        # ---- Phase C: output projection -------------------------------
        with tc.tile_pool(name="yo", bufs=3) as ypool, \
             tc.tile_pool(name="psC", bufs=4, space="PSUM") as psC:
            for qb in range(NQB):
                ysb = ypool.tile([P, D], F32, tag="y")
                for oh in range(2):
                    yp = psC.tile([P, NS], F32, tag="yp")
                    for ct in range(2):
                        nc.tensor.matmul(
                            yp[:],
                            cT_sb[:, ct, qb * P:(qb + 1) * P],
                            wo_sb[:, ct, oh * NS:(oh + 1) * NS],
                            start=(ct == 0),
                            stop=(ct == 1),
                        )
                    nc.vector.tensor_copy(ysb[:, oh * NS:(oh + 1) * NS], yp[:])
                nc.sync.dma_start(y.ap()[qb * P:(qb + 1) * P, :], ysb[:])

    nc.compile()
    return nc


_NC = None


def _get_nc():
    global _NC
    if _NC is None:
        _NC = build_nc()
    return _NC


def _shard_inputs(Query, Key, Value, W_q, W_k, W_v, W_o):
    in_maps = []
    xT = {}
    for b in range(B):
        xT[b] = (
            np.ascontiguousarray(Query[b].T),
            np.ascontiguousarray(Key[b].T),
            np.ascontiguousarray(Value[b].T),
        )
    for b in range(B):
        for hg in range(4):
            r0 = hg * CL
            in_maps.append({
                "xqT": xT[b][0],
                "xkT": xT[b][1],
                "xvT": xT[b][2],
                "wqT": np.ascontiguousarray(W_q[r0:r0 + CL, :].T),
                "wkT": np.ascontiguousarray(W_k[r0:r0 + CL, :].T),
                "wvT": np.ascontiguousarray(W_v[r0:r0 + CL, :].T),
                "woT": np.ascontiguousarray(W_o[:, r0:r0 + CL].T),
            })
    return in_maps


def _reference_np(Query, Key, Value, mask, W_q, W_k, W_v, W_o):
    # Fallback for a non-trivial mask (never hit for the spec'd inputs).
    out = np.empty((B, S, D), dtype=np.float32)
    m = np.broadcast_to(mask, (1, 1, S, S))[0, 0]
    for b in range(B):
        Q = (Query[b] @ W_q.T).reshape(S, H, DK).transpose(1, 0, 2)
        K = (Key[b] @ W_k.T).reshape(S, H, DK).transpose(1, 0, 2)
        V = (Value[b] @ W_v.T).reshape(S, H, DK).transpose(1, 0, 2)
        ctx = np.empty((H, S, DK), dtype=np.float32)
        for h in range(H):
            s = (Q[h] @ K[h].T) / np.sqrt(DK)
            s = np.where(m == 0, -1e9, s)
            s -= s.max(axis=-1, keepdims=True)
            e = np.exp(s)
            ctx[h] = (e / e.sum(axis=-1, keepdims=True)) @ V[h]
        out[b] = ctx.transpose(1, 0, 2).reshape(S, D) @ W_o.T
    return out


def kernel(Query, Key, Value, mask, W_q, W_k, W_v, W_o, **_ignored):
    Query = np.asarray(Query, dtype=np.float32)
    Key = np.asarray(Key, dtype=np.float32)
    Value = np.asarray(Value, dtype=np.float32)
    W_q = np.asarray(W_q, dtype=np.float32)
    W_k = np.asarray(W_k, dtype=np.float32)
    W_v = np.asarray(W_v, dtype=np.float32)
    W_o = np.asarray(W_o, dtype=np.float32)

    if not np.all(np.asarray(mask) != 0):
        return _reference_np(Query, Key, Value, np.asarray(mask),
                             W_q, W_k, W_v, W_o)

    nc = _get_nc()
    in_maps = _shard_inputs(Query, Key, Value, W_q, W_k, W_v, W_o)
    res = bass_utils.run_bass_kernel_spmd(nc, in_maps, core_ids=list(range(8)))
    out = np.zeros((B, S, D), dtype=np.float32)
    for b in range(B):
        for hg in range(4):
            out[b] += res.results[b * 4 + hg]["y"]
    return out


# revision 5
# speedup vs baseline: 1.3934x; 1.3934x over previous
"""Multi-head attention forward on 8 Trainium2 NeuronCores.

Sharding: core = (batch b in 0..2, head-group hg in 0..4); each core owns
4 of the 16 heads for one batch element. Q/K/V projections are computed
per-core for its 256 head-dims; attention runs per head with scores kept
transposed (S^T[k, q]) so no on-chip transposes are needed; the output
projection is row-sharded over W_o, producing a per-core partial Y that
the host sums over the 4 head-groups of each batch.

All matmul operands are fp16 (PSUM accumulation is fp32): 1 cycle/row on
the PE and FWL-hidden weight loads, vs ~2x slower fp32r with serialized
LDWEIGHTS. Softmax denominators come free from a ones column appended to
V; the PV stationary is padded to 128 columns to keep FWL eligible.
"""

import sys

for _p in ("/opt/trn_rl_repo", "/opt/pypackages"):
    if _p not in sys.path:
        sys.path.append(_p)

from contextlib import ExitStack

import numpy as np

import concourse.bass as bass
import concourse.tile as tile
from concourse import bacc, mybir
from concourse import bass_utils

P = 128
B = 2
S = 2048          # sequence length
D = 1024          # model dim
H = 16            # total heads
DK = 64           # head dim
HL = 4            # heads per core
CL = HL * DK      # local head dims per core (256)
NJ = 4            # 512-wide s-slices
NS = 512
NI = D // P       # 8 contraction tiles over model dim
NK = S // P       # 16 key tiles
NQB = S // P      # 16 query blocks for the output projection
VW = 65           # V columns per head (64 dims + ones)
VPAD = HL * VW + 63  # pad so a 128-wide lhsT slice exists for every head

F32 = mybir.dt.float32
F16 = mybir.dt.float16
EXP = mybir.ActivationFunctionType.Exp

# k-tile groups per (head, q-slice): scores for a group land in one PSUM
# tile and get one big exp instruction ([128, len*512] PSUM -> SBUF).
K_GROUPS = [(0, 1, 2), (3, 4, 5), (6, 7, 8), (9, 10, 11), (12, 13, 14), (15,)]


def build_nc():
    nc = bacc.Bacc("TRN2", target_bir_lowering=False, debug=False)

    xqT = nc.dram_tensor("xqT", [D, S], F16, kind="ExternalInput")
    xkT = nc.dram_tensor("xkT", [D, S], F16, kind="ExternalInput")
    xvT = nc.dram_tensor("xvT", [D, S], F16, kind="ExternalInput")
    wqT = nc.dram_tensor("wqT", [D, CL], F16, kind="ExternalInput")
    wkT = nc.dram_tensor("wkT", [D, CL], F16, kind="ExternalInput")
    wvT = nc.dram_tensor("wvT", [D, CL], F16, kind="ExternalInput")
    woT = nc.dram_tensor("woT", [CL, D], F16, kind="ExternalInput")
    y = nc.dram_tensor("y", [S, D], F32, kind="ExternalOutput")

    with tile.TileContext(nc) as tc, ExitStack() as ctx:
        wpool = ctx.enter_context(tc.tile_pool(name="w", bufs=1))
        big = ctx.enter_context(tc.tile_pool(name="big", bufs=1))

        # Resident weights
        wq_sb = wpool.tile([P, NI, CL], F16)
        wk_sb = wpool.tile([P, NI, CL], F16)
        wv_sb = wpool.tile([P, NI, CL], F16)
        wo_sb = wpool.tile([P, CL // P, D], F16)
        nc.sync.dma_start(wq_sb[:], wqT.ap().rearrange("(n p) o -> p n o", p=P))
        nc.sync.dma_start(wk_sb[:], wkT.ap().rearrange("(n p) o -> p n o", p=P))
        nc.sync.dma_start(wv_sb[:], wvT.ap().rearrange("(n p) o -> p n o", p=P))
        nc.sync.dma_start(wo_sb[:], woT.ap().rearrange("(n p) o -> p n o", p=P))

        # Resident activations: Q^T / K^T with head dims on partitions
        # ([128, o-tile, s]); V in natural [s, d] layout with a ones column
        # appended per head (softmax denominators); normalized ctx^T.
        qT_sb = big.tile([P, 2, S], F16)
        kT_sb = big.tile([P, 2, S], F16)
        v_sb = big.tile([P, NK, VPAD], F16)
        cT_sb = big.tile([P, 2, S], F16)

        nc.vector.memset(v_sb[:], 1.0)

        # ---- Phase A: projections -------------------------------------
        with tc.tile_pool(name="xs", bufs=12) as xpool, \
             tc.tile_pool(name="psA", bufs=2, space="PSUM") as psA, \
             tc.tile_pool(name="psV", bufs=2, space="PSUM") as psV:

            def proj_T(x_dram, w_sb, out_sb):
                # out_sb[:, ot, j*512:...] = (X @ W.T)^T for this core's dims
                for j in range(NJ):
                    xt = []
                    for i in range(NI):
                        t = xpool.tile([P, NS], F16, tag="x")
                        nc.sync.dma_start(
                            t[:], x_dram.ap()[i * P:(i + 1) * P, j * NS:(j + 1) * NS],
                        )
                        xt.append(t)
                    for ot in range(2):
                        ps = psA.tile([P, NS], F32, tag="ps")
                        for i in range(NI):
                            nc.tensor.matmul(
                                ps[:],
                                w_sb[:, i, ot * P:(ot + 1) * P],
                                xt[i][:],
                                start=(i == 0),
                                stop=(i == NI - 1),
                            )
                        nc.scalar.copy(out_sb[:, ot, j * NS:(j + 1) * NS], ps[:])

            proj_T(xkT, wk_sb, kT_sb)

            # V in natural layout: out[s_tile] = X_v @ W_v.T
            for j in range(NJ):
                xt = []
                for i in range(NI):
                    t = xpool.tile([P, NS], F16, tag="x")
                    nc.sync.dma_start(
                        t[:], xvT.ap()[i * P:(i + 1) * P, j * NS:(j + 1) * NS],
                    )
                    xt.append(t)
                for sb in range(4):
                    psv = psV.tile([P, CL], F32, tag="psv")
                    for i in range(NI):
                        nc.tensor.matmul(
                            psv[:],
                            xt[i][:, sb * P:(sb + 1) * P],
                            wv_sb[:, i, :],
                            start=(i == 0),
                            stop=(i == NI - 1),
                        )
                    st = j * 4 + sb
                    nc.vector.tensor_copy(
                        v_sb[:, st, 0:HL * VW].rearrange("p (h c) -> p h c", c=VW)[:, :, 0:64],
                        psv[:].rearrange("p (h c) -> p h c", c=64),
                    )

            proj_T(xqT, wq_sb, qT_sb)

        # ---- Phase B: attention ---------------------------------------
        with tc.tile_pool(name="ex", bufs=3) as epool, \
             tc.tile_pool(name="sm", bufs=3) as spool, \
             tc.tile_pool(name="psS", bufs=2, space="PSUM") as psS, \
             tc.tile_pool(name="psB", bufs=2, space="PSUM") as psB:
            for h in range(HL):
                pr0 = (h % 2) * 64
                ot = h // 2
                vcol = h * VW
                for j in range(NJ):
                    # rows 0:64 = unnormalized ctx^T, row 64 = sum(exp),
                    # rows 65:128 = don't-care (128-wide lhsT for FWL)
                    ctx_ps = psB.tile([P, NS], F32, tag="ctx")
                    for ks in K_GROUPS:
                        ng = len(ks)
                        sp = psS.tile([P, 3, NS], F32, tag="sc")
                        for idx, k in enumerate(ks):
                            nc.tensor.matmul(
                                sp[:, idx],
                                kT_sb[pr0:pr0 + 64, ot, k * P:(k + 1) * P],
                                qT_sb[pr0:pr0 + 64, ot, j * NS:(j + 1) * NS],
                                start=True,
                                stop=True,
                            )
                        ex = epool.tile([P, 3, NS], F16, tag="ex")
                        nc.scalar.activation(
                            ex[:, 0:ng], sp[:, 0:ng], EXP, scale=0.125,
                        )
                        for idx, k in enumerate(ks):
                            nc.tensor.matmul(
                                ctx_ps[:],
                                v_sb[:, k, vcol:vcol + P],
                                ex[:, idx],
                                start=(k == 0),
                                stop=(k == NK - 1),
                            )
                    # Plain-DVE copy first: custom DVE ops cannot shift
                    # partitions, so align the denominator to partition 0.
                    den = spool.tile([1, NS], F32, tag="den")
                    nc.vector.tensor_copy(den[:], ctx_ps[64:65, :])
                    rec = spool.tile([1, NS], F32, tag="rec")
                    nc.vector.reciprocal_approx_fast(rec[:], den[:])
                    bc = spool.tile([64, NS], F32, tag="bc")
                    nc.gpsimd.partition_broadcast(bc[:], rec[:])
                    dst = cT_sb[pr0:pr0 + 64, ot, j * NS:(j + 1) * NS]
                    if h % 2 == 0:
                        nc.vector.tensor_mul(dst, ctx_ps[0:64, :], bc[:])
                    else:
                        tmp = spool.tile([64, NS], F16, tag="tmp")
                        nc.vector.tensor_mul(tmp[:], ctx_ps[0:64, :], bc[:])
                        nc.sync.dma_start(dst, tmp[:])

        # ---- Phase C: output projection -------------------------------
        with tc.tile_pool(name="yo", bufs=3) as ypool, \
             tc.tile_pool(name="psC", bufs=4, space="PSUM") as psC:
            for qb in range(NQB):
                ysb = ypool.tile([P, D], F32, tag="y")
                for oh in range(2):
                    yp = psC.tile([P, NS], F32, tag="yp")
                    for ct in range(2):
                        nc.tensor.matmul(
                            yp[:],
                            cT_sb[:, ct, qb * P:(qb + 1) * P],
                            wo_sb[:, ct, oh * NS:(oh + 1) * NS],
                            start=(ct == 0),
                            stop=(ct == 1),
                        )
                    nc.vector.tensor_copy(ysb[:, oh * NS:(oh + 1) * NS], yp[:])
                nc.sync.dma_start(y.ap()[qb * P:(qb + 1) * P, :], ysb[:])

    nc.compile()
    return nc


_NC = None


def _get_nc():
    global _NC
    if _NC is None:
        _NC = build_nc()
    return _NC


def _shard_inputs(Query, Key, Value, W_q, W_k, W_v, W_o):
    in_maps = []
    xT = {}
    for b in range(B):
        xT[b] = (
            np.ascontiguousarray(Query[b].T).astype(np.float16),
            np.ascontiguousarray(Key[b].T).astype(np.float16),
            np.ascontiguousarray(Value[b].T).astype(np.float16),
        )
    for b in range(B):
        for hg in range(4):
            r0 = hg * CL
            in_maps.append({
                "xqT": xT[b][0],
                "xkT": xT[b][1],
                "xvT": xT[b][2],
                "wqT": np.ascontiguousarray(W_q[r0:r0 + CL, :].T).astype(np.float16),
                "wkT": np.ascontiguousarray(W_k[r0:r0 + CL, :].T).astype(np.float16),
                "wvT": np.ascontiguousarray(W_v[r0:r0 + CL, :].T).astype(np.float16),
                "woT": np.ascontiguousarray(W_o[:, r0:r0 + CL].T).astype(np.float16),
            })
    return in_maps


def _reference_np(Query, Key, Value, mask, W_q, W_k, W_v, W_o):
    # Fallback for a non-trivial mask (never hit for the spec'd inputs).
    out = np.empty((B, S, D), dtype=np.float32)
    m = np.broadcast_to(mask, (1, 1, S, S))[0, 0]
    for b in range(B):
        Q = (Query[b] @ W_q.T).reshape(S, H, DK).transpose(1, 0, 2)
        K = (Key[b] @ W_k.T).reshape(S, H, DK).transpose(1, 0, 2)
        V = (Value[b] @ W_v.T).reshape(S, H, DK).transpose(1, 0, 2)
        ctx = np.empty((H, S, DK), dtype=np.float32)
        for h in range(H):
            s = (Q[h] @ K[h].T) / np.sqrt(DK)
            s = np.where(m == 0, -1e9, s)
            s -= s.max(axis=-1, keepdims=True)
            e = np.exp(s)
            ctx[h] = (e / e.sum(axis=-1, keepdims=True)) @ V[h]
        out[b] = ctx.transpose(1, 0, 2).reshape(S, D) @ W_o.T
    return out


def kernel(Query, Key, Value, mask, W_q, W_k, W_v, W_o, **_ignored):
    Query = np.asarray(Query, dtype=np.float32)
    Key = np.asarray(Key, dtype=np.float32)
    Value = np.asarray(Value, dtype=np.float32)
    W_q = np.asarray(W_q, dtype=np.float32)
    W_k = np.asarray(W_k, dtype=np.float32)
    W_v = np.asarray(W_v, dtype=np.float32)
    W_o = np.asarray(W_o, dtype=np.float32)

    if not np.all(np.asarray(mask) != 0):
        return _reference_np(Query, Key, Value, np.asarray(mask),
                             W_q, W_k, W_v, W_o)

    nc = _get_nc()
    in_maps = _shard_inputs(Query, Key, Value, W_q, W_k, W_v, W_o)
    res = bass_utils.run_bass_kernel_spmd(nc, in_maps, core_ids=list(range(8)))
    out = np.zeros((B, S, D), dtype=np.float32)
    for b in range(B):
        for hg in range(4):
            out[b] += res.results[b * 4 + hg]["y"]
    return out
